# revision 19
# baseline (speedup 1.0000x reference)
"""Trainium2 Bass kernel for nn_DeconvDft2dLayer.

y = irfft2(gmf * rfft2(pad(x)))  with x (64,512,512), w (3,3), y (64,768,768).

Strategy: data-parallel over batch (8 samples per NeuronCore). Per sample the
FFTs are evaluated as DFT matmuls on the tensor engine (fp32r, full rate):

  A : S1^T[w,k] = sum_h x[h,w] W2[h,k]            k in [0,385)   (fft-H, halved
      via Hermitian symmetry of the real input)
  B : X[k,j]    = sum_w S1[k,w] C1[w,j]  (Karatsuba: M1=S1re@C1re, M2=S1im@C1im,
      M3=(S1re+S1im)@(C1re+-C1im); re=M1-+M2, im=M3-re-+2*M2; mirror chunks use
      conj(C1) which flips the +- signs)
  C : T[r,j] = gmf[rho(r)] * X  -- elementwise on Vector+GpSimd
      (rho(r) = r for r<384, 1151-r otherwise)
  D : U^T[j,n]  = sum_r T[r,j] Atil[r,n]  (Karatsuba again)
  E : y[n,m]    = sum_j Ure[j,n] Bre[j,m] + Uim[j,n] Bimn[j,m]
      Bre = w_j cos(2 pi j m/768), Bimn = -w_j sin(2 pi j m/768)

Nyquist handling (no dedicated matmuls): column j=384 of the pq table holds the
r-Hermitian-symmetrized filter, so T[:,384] is Hermitian and ifft(T[:,384]) =
Re(U[:,384]).  After stage C the j=0 column (whose U is real since the j=0
filter column is Hermitian) is mixed as Tmix = T[:,0] + i*T[:,384]; stage D then
lands U[:,0] in Ure row 0 and U384re in the (otherwise zero) Uim row 0, and the
Bimn j=0 row is host-set to (-1)^m so stage E adds the Nyquist term for free.

gmf and the DFT matrices are tiny 3x3-derived constants computed host-side
(float64) and replicated to all cores; no cross-device communication.
All DRAM tensors are host-packed in the exact SBUF tile layout so every DMA is
128 large contiguous descriptors.
"""
import os

import ml_dtypes
import numpy as np

import concourse.bacc as bacc
import concourse.mybir as mybir
import concourse.tile as tile
from concourse.bass_utils import run_bass_kernel_spmd

F32 = mybir.dt.float32
F32R = mybir.dt.float32r
BF16 = mybir.dt.bfloat16

HP = 768          # padded grid
J = 385           # rfft half length (768//2+1)
JP = 386          # padded to even for fp32r free-dim constraint
NS = 8            # samples per core
NCORES = 8

LAST_EXEC_NS = None
LAST_RESULTS = None


def _build_constants(w):
    """Host-side constants (float64 -> float32), packed in SBUF tile layout."""
    w = np.asarray(w, np.float64)
    hm1 = np.zeros((HP, HP)); hm1[:3, :3] = w
    gm1f = 1.0 / np.fft.rfft2(hm1)
    gm2f = np.roll(gm1f[::-1, :], shift=1, axis=0)
    gm3f = np.roll(gm1f[:, ::-1], shift=1, axis=1)
    gm4f = np.roll(gm3f[::-1, :], shift=1, axis=0)
    gmf = (gm1f * gm2f) * (gm3f * gm4f)          # (768, 385) complex

    h = np.arange(512)
    k = np.arange(J)
    ph = np.exp(-2j * np.pi * (np.outer(h + 128, k) % HP) / HP)   # (512,385)
    c1 = np.zeros((4, 512, JP))
    c1[0, :, :J] = ph.real            # C1 == W2 (same 512x385 phase table)
    c1[1, :, :J] = ph.imag
    c1[2] = c1[0] + c1[1]             # Karatsuba sum table
    c1[3] = c1[0] - c1[1]             # Karatsuba dif table (mirror chunks)

    # pq with IDENTITY row map (layout row r holds true spectral row r; the
    # mirror chunks of stage B are emitted k'-reversed to make this so)
    r = np.arange(HP)
    pq = np.zeros((2, HP, JP))
    pq[0, :, :J] = gmf.real
    pq[1, :, :J] = gmf.imag
    # Nyquist pack: j=384 column becomes the r-symmetrized filter so that
    # T[:,384] is Hermitian in r and ifft of it is real (= U384re).
    gsym = (gmf[:, 384] + np.conj(gmf[(HP - r) % HP, 384])) / 2.0
    pq[0, :, 384] = gsym.real
    pq[1, :, 384] = gsym.imag

    # radix-2 DIF split tables for stage D: U[2n'] = sum_s (T[s]+T[s+384]) ae,
    # U[2n'+1] = sum_s (T[s]-T[s+384]) ao
    s = np.arange(384)
    npr = np.arange(384)
    ae = np.exp(2j * np.pi * np.outer(s, npr) / 384.0) / (HP * HP)
    ao = np.exp(2j * np.pi * (np.outer(s, 2 * npr + 1) % HP) / HP) / (HP * HP)
    a2 = np.zeros((3, 2, 384, 384))
    for se, tab in enumerate((ae, ao)):
        a2[0, se] = tab.real
        a2[1, se] = tab.imag
        a2[2, se] = tab.real + tab.imag

    m = np.arange(HP)
    j = np.arange(J)
    wj = np.where((j == 0) | (j == 384), 1.0, 2.0)
    ang = 2 * np.pi * (np.outer(j, m) % HP) / HP
    bre = wj[:, None] * np.cos(ang)              # (385, 768)
    bimn = -wj[:, None] * np.sin(ang)
    bmat = np.stack([bre[:384], bimn[:384]])     # (2, 384, 768)
    bmat[1, 0, :] = np.cos(np.pi * m)            # packed Nyquist row: (-1)^m

    f = np.float32
    bf = ml_dtypes.bfloat16
    return {
        # packed to SBUF layouts: leading dim = partition
        "c1": np.ascontiguousarray(c1.reshape(4, 4, 128, JP).transpose(2, 0, 1, 3)).astype(bf),
        "pq": np.ascontiguousarray(pq.reshape(2, 6, 128, JP).transpose(2, 0, 1, 3)).astype(bf),
        "a2": np.ascontiguousarray(a2.reshape(3, 2, 3, 128, 384).transpose(3, 0, 1, 2, 4)).astype(bf),
        "bmat": np.ascontiguousarray(bmat.reshape(2, 3, 128, HP).transpose(2, 0, 1, 3)).astype(bf),
    }


def _build_program(ns=NS):
    nc = bacc.Bacc("TRN2", target_bir_lowering=False, debug=False,
                   num_devices=NCORES)
    x_ext = nc.declare_dram_parameter("x", [ns, 128, 4, 512], BF16, isOutput=False)
    y_ext = nc.declare_dram_parameter("y", [ns, 128, 6, HP], F32, isOutput=True)
    c1_ext = nc.declare_dram_parameter("c1", [128, 4, 4, JP], BF16, isOutput=False)
    pq_ext = nc.declare_dram_parameter("pq", [128, 2, 6, JP], BF16, isOutput=False)
    a2_ext = nc.declare_dram_parameter("a2", [128, 3, 2, 3, 384], BF16, isOutput=False)
    bmat_ext = nc.declare_dram_parameter("bmat", [128, 2, 3, HP], BF16, isOutput=False)

    MUL = mybir.AluOpType.mult
    ADD = mybir.AluOpType.add
    SUB = mybir.AluOpType.subtract

    TRE, TIM, TSUM = 0, 1, 2

    with tile.TileContext(nc) as tc:
        with tc.tile_pool(name="const", bufs=1) as cpool, \
             tc.tile_pool(name="data", bufs=2) as dpool, \
             tc.tile_pool(name="xin", bufs=1) as xpool, \
             tc.tile_pool(name="yout", bufs=2) as ypool, \
             tc.tile_pool(name="scr", bufs=2) as spool, \
             tc.tile_pool(name="psA", bufs=2, space="PSUM") as ppa, \
             tc.tile_pool(name="psBD", bufs=4, space="PSUM") as ppb, \
             tc.tile_pool(name="psE", bufs=2, space="PSUM") as ppe:

            # sample-0 input first so stage A can start during const loads
            xts = []
            xt0 = xpool.tile([128, 4, 512], BF16, tag="x")
            nc.sync.dma_start(out=xt0[:], in_=x_ext[0])
            xts.append(xt0)

            c1_t = cpool.tile([128, 4, 4, JP], BF16, tag="c1")
            nc.sync.dma_start(out=c1_t[:, 0:1], in_=c1_ext[:, 0:1])
            nc.sync.dma_start(out=c1_t[:, 1:2], in_=c1_ext[:, 1:2])
            nc.sync.dma_start(out=c1_t[:, 2:4], in_=c1_ext[:, 2:4])
            pq_t = cpool.tile([128, 2, 6, JP], BF16, tag="pq")
            nc.sync.dma_start(out=pq_t[:], in_=pq_ext[:])
            a2_t = cpool.tile([128, 3, 2, 3, 384], BF16, tag="a2")
            nc.sync.dma_start(out=a2_t[:], in_=a2_ext[:])
            b_t = cpool.tile([128, 2, 3, HP], BF16, tag="bmat")
            nc.sync.dma_start(out=b_t[:], in_=bmat_ext[:])

            def mm(ps, lhsT, rhs, start, stop):
                nc.tensor.matmul(ps, lhsT=lhsT, rhs=rhs, start=start, stop=stop)

            def stage_e(eb, eut):
                for nch in range(6):
                    nsl = slice(nch * 128, nch * 128 + 128)
                    ytc = ypool.tile([128, HP], F32, tag="y")
                    for mh in range(2):
                        msl = slice(mh * 384, mh * 384 + 384)
                        ps_y = ppe.tile([128, 384], F32, tag="psE")
                        for jc in range(3):
                            mm(ps_y[:], eut[:, 0, jc, nsl], b_t[:, 0, jc, msl], jc == 0, False)
                        for jc in range(3):
                            mm(ps_y[:], eut[:, 1, jc, nsl], b_t[:, 1, jc, msl], False, jc == 2)
                        nc.scalar.copy(out=ytc[:, msl], in_=ps_y[:])
                        nc.sync.dma_start(out=y_ext[eb, :, nch, msl], in_=ytc[:, msl])

            prev_e = None
            for b in range(ns):
                xt = xts[b]
                if b + 1 < ns:   # prefetch next sample
                    nxt = xpool.tile([128, 4, 512], BF16, tag="x")
                    nc.sync.dma_start(out=nxt[:], in_=x_ext[b + 1])
                    xts.append(nxt)

                s1 = dpool.tile([128, 3, 4, JP], BF16, tag="s1")
                s1r = dpool.tile([128, 3, 4, 384], BF16, tag="s1r")
                tmat = dpool.tile([128, 2, 6, JP], BF16, tag="tmat")
                vmat = dpool.tile([128, 3, 2, 3, JP], BF16, tag="vmat")
                ut = dpool.tile([128, 2, 3, HP], BF16, tag="ut")

                # ---- stage A ----
                for comp in range(2):
                    for wc in range(4):
                        ps = ppa.tile([128, JP], F32, tag="psA")
                        for hc in range(4):
                            mm(ps[:], xt[:, hc, wc * 128:(wc + 1) * 128],
                               c1_t[:, comp, hc, :], hc == 0, hc == 3)
                        nc.scalar.copy(out=s1[:, comp, wc, :], in_=ps[:])
                        # reversed copy for the R-ascending mirror chunks:
                        # s1r[., i] = S1[., 384-i]
                        nc.scalar.copy(out=s1r[:, comp, wc, :], in_=ps[:, 384:0:-1])
                # Karatsuba lhsT sums (per w-chunk so B's M3 can start early)
                for wc in range(4):
                    nc.gpsimd.tensor_tensor(out=s1[:, 2, wc, :], in0=s1[:, 0, wc, :],
                                            in1=s1[:, 1, wc, :], op=ADD)
                    nc.gpsimd.tensor_tensor(out=s1r[:, 2, wc, :], in0=s1r[:, 0, wc, :],
                                            in1=s1r[:, 1, wc, :], op=ADD)

                # ---- stages B + C, 6 chunks of T ----
                for c in range(6):
                    mirror = c >= 3
                    sm = s1r if mirror else s1
                    ksl = slice((c - 3) * 128, (c - 3) * 128 + 128) if mirror \
                        else slice(c * 128, c * 128 + 128)
                    ps1 = ppb.tile([128, JP], F32, tag="psBD")
                    ps2 = ppb.tile([128, JP], F32, tag="psBD")
                    ps3 = ppb.tile([128, JP], F32, tag="psBD")
                    for wc in range(4):
                        mm(ps1[:], sm[:, 0, wc, ksl], c1_t[:, 0, wc, :], wc == 0, wc == 3)
                    for wc in range(4):
                        mm(ps2[:], sm[:, 1, wc, ksl], c1_t[:, 1, wc, :], wc == 0, wc == 3)
                    for wc in range(4):
                        mm(ps3[:], sm[:, 2, wc, ksl],
                           c1_t[:, 3 if mirror else 2, wc, :], wc == 0, wc == 3)
                    # Karatsuba recombine: xre = M1 -+ M2 ; xim = M3 - M1 -+ M2
                    # (M2 staged through ACT since DVE reads only one PSUM input)
                    xre = spool.tile([128, JP], BF16, tag="xre")
                    xim = spool.tile([128, JP], BF16, tag="xim")
                    nc.scalar.copy(out=xim[:], in_=ps2[:])
                    nc.vector.tensor_tensor(out=xre[:], in0=ps1[:], in1=xim[:],
                                            op=ADD if mirror else SUB)
                    nc.vector.scalar_tensor_tensor(
                        out=xim[:], in0=xim[:], scalar=-2.0 if mirror else 2.0,
                        in1=xre[:], op0=MUL, op1=ADD)
                    nc.vector.tensor_tensor(out=xim[:], in0=ps3[:], in1=xim[:],
                                            op=SUB)

                    # ---- stage C on this chunk ----
                    t_re = tmat[:, TRE, c, :]
                    t_im = tmat[:, TIM, c, :]
                    pA = spool.tile([128, JP], BF16, tag="pA")
                    pB = spool.tile([128, JP], BF16, tag="pB")
                    nc.vector.tensor_tensor(out=t_re, in0=pq_t[:, 0, c, :], in1=xre[:], op=MUL)
                    nc.vector.tensor_tensor(out=pA[:], in0=pq_t[:, 1, c, :], in1=xim[:], op=MUL)
                    nc.vector.tensor_tensor(out=t_re, in0=t_re, in1=pA[:],
                                            op=ADD if mirror else SUB)
                    nc.vector.tensor_tensor(out=t_im, in0=pq_t[:, 1, c, :], in1=xre[:], op=MUL)
                    nc.vector.tensor_tensor(out=pB[:], in0=pq_t[:, 0, c, :], in1=xim[:], op=MUL)
                    nc.vector.tensor_tensor(out=t_im, in0=t_im, in1=pB[:],
                                            op=SUB if mirror else ADD)
                    # Tmix: pack Nyquist column into j=0 (per-chunk so stage D
                    # pipelines per tmat chunk instead of waiting for all of C)
                    nc.vector.tensor_tensor(out=t_re[:, 0:1], in0=t_re[:, 0:1],
                                            in1=t_im[:, 384:385], op=SUB)
                    nc.vector.tensor_tensor(out=t_im[:, 0:1], in0=t_im[:, 0:1],
                                            in1=t_re[:, 384:385], op=ADD)
                    # radix-2 DIF fold once the (sc, sc+3) chunk pair is done:
                    # V+[s] = T[s] + T[s+384], V-[s] = T[s] - T[s+384]
                    if mirror:
                        sc = c - 3
                        for cmp in range(2):
                            nc.vector.tensor_tensor(
                                out=vmat[:, cmp, 0, sc, :], in0=tmat[:, cmp, sc, :],
                                in1=tmat[:, cmp, c, :], op=ADD)
                            nc.vector.tensor_tensor(
                                out=vmat[:, cmp, 1, sc, :], in0=tmat[:, cmp, sc, :],
                                in1=tmat[:, cmp, c, :], op=SUB)
                        for se in range(2):
                            nc.gpsimd.tensor_tensor(
                                out=vmat[:, 2, se, sc, :], in0=vmat[:, 0, se, sc, :],
                                in1=vmat[:, 1, se, sc, :], op=ADD)

                # stage E of the previous sample goes here in program order so
                # the PE has ready matmuls while this sample's C chains drain
                if prev_e is not None:
                    stage_e(*prev_e)

                # ---- stage D: radix-2 split, 9 matmuls per (jc, parity) ----
                for jc in range(3):
                    jsl = slice(jc * 128, jc * 128 + 128)
                    for se in range(2):
                        nsl = slice(se * 384, se * 384 + 384)
                        pm1 = ppb.tile([128, 384], F32, tag="psBD")
                        pm2 = ppb.tile([128, 384], F32, tag="psBD")
                        pm3 = ppb.tile([128, 384], F32, tag="psBD")
                        for sc in range(3):
                            mm(pm1[:], vmat[:, 0, se, sc, jsl], a2_t[:, 0, se, sc, :], sc == 0, sc == 2)
                        for sc in range(3):
                            mm(pm2[:], vmat[:, 1, se, sc, jsl], a2_t[:, 1, se, sc, :], sc == 0, sc == 2)
                        for sc in range(3):
                            mm(pm3[:], vmat[:, 2, se, sc, jsl], a2_t[:, 2, se, sc, :], sc == 0, sc == 2)
                        # Ure = M1 - M2 ; Uim = M3 - M1 - M2
                        # (M2 staged into the Uim slot via ACT; one-PSUM rule)
                        u0 = ut[:, 0, jc, nsl]
                        u1 = ut[:, 1, jc, nsl]
                        nc.scalar.copy(out=u1, in_=pm2[:])
                        nc.vector.tensor_tensor(out=u0, in0=pm1[:], in1=u1, op=SUB)
                        nc.vector.scalar_tensor_tensor(
                            out=u1, in0=u1, scalar=2.0, in1=u0, op0=MUL, op1=ADD)
                        nc.vector.tensor_tensor(out=u1, in0=pm3[:], in1=u1, op=SUB)

                prev_e = (b, ut)

            stage_e(*prev_e)

    nc.compile()
    return nc


_PROGRAM_CACHE = {}


def kernel(x, w, trace=False):
    global LAST_EXEC_NS, LAST_RESULTS
    x = np.asarray(x, np.float32)
    B = x.shape[0]
    # pack to the SBUF tile layout: x_dev[b, p, c, w] = x[b, c*128+p, w]
    x_dev = np.ascontiguousarray(
        x.reshape(B, 4, 128, 512).transpose(0, 2, 1, 3)).astype(ml_dtypes.bfloat16)
    consts = _build_constants(w)
    if NS not in _PROGRAM_CACHE:
        _PROGRAM_CACHE[NS] = _build_program(NS)
    nc = _PROGRAM_CACHE[NS]
    in_maps = []
    for core in range(NCORES):
        m = {"x": x_dev[core * NS:(core + 1) * NS]}
        m.update(consts)
        in_maps.append(m)
    if trace:
        os.environ.pop("BASS_NEVER_TRACE", None)
        res = run_bass_kernel_spmd(nc, in_maps, list(range(NCORES)), trace=True)
    else:
        # profiling needs the antenv NTFF shim; never let a stray BASS_TRACE
        # env var route us down that path during plain runs
        os.environ["BASS_NEVER_TRACE"] = "1"
        try:
            res = run_bass_kernel_spmd(nc, in_maps, list(range(NCORES)), trace=False)
        finally:
            os.environ.pop("BASS_NEVER_TRACE", None)
    LAST_EXEC_NS = res.exec_time_ns
    LAST_RESULTS = res
    # unshard: y_dev[b, p, c, m] -> row q = c*128+p; stage D's radix-2 DIF
    # interleave means q < 384 holds even output rows, q >= 384 odd rows
    y_dev = np.concatenate([res.results[i]["y"] for i in range(NCORES)], axis=0)
    yq = y_dev.transpose(0, 2, 1, 3).reshape(B, HP, HP)
    q = np.arange(HP)
    perm = np.where(q < 384, 2 * q, 2 * (q - 384) + 1)
    y = np.empty_like(yq)
    y[:, perm, :] = yq
    return np.ascontiguousarray(y, np.float32)


# revision 20
# speedup vs baseline: 1.0971x; 1.0971x over previous
"""Trainium2 Bass kernel for nn_DeconvDft2dLayer.

y = irfft2(gmf * rfft2(pad(x)))  with x (64,512,512), w (3,3), y (64,768,768).

Strategy: data-parallel over batch (8 samples per NeuronCore). Per sample the
FFTs are evaluated as DFT matmuls on the tensor engine (fp32r, full rate):

  A : S1^T[w,k] = sum_h x[h,w] W2[h,k]            k in [0,385)   (fft-H, halved
      via Hermitian symmetry of the real input)
  B : X[k,j]    = sum_w S1[k,w] C1[w,j]  (Karatsuba: M1=S1re@C1re, M2=S1im@C1im,
      M3=(S1re+S1im)@(C1re+-C1im); re=M1-+M2, im=M3-re-+2*M2; mirror chunks use
      conj(C1) which flips the +- signs)
  C : T[r,j] = gmf[rho(r)] * X  -- elementwise on Vector+GpSimd
      (rho(r) = r for r<384, 1151-r otherwise)
  D : U^T[j,n]  = sum_r T[r,j] Atil[r,n]  (Karatsuba again)
  E : y[n,m]    = sum_j Ure[j,n] Bre[j,m] + Uim[j,n] Bimn[j,m]
      Bre = w_j cos(2 pi j m/768), Bimn = -w_j sin(2 pi j m/768)

Nyquist handling (no dedicated matmuls): column j=384 of the pq table holds the
r-Hermitian-symmetrized filter, so T[:,384] is Hermitian and ifft(T[:,384]) =
Re(U[:,384]).  After stage C the j=0 column (whose U is real since the j=0
filter column is Hermitian) is mixed as Tmix = T[:,0] + i*T[:,384]; stage D then
lands U[:,0] in Ure row 0 and U384re in the (otherwise zero) Uim row 0, and the
Bimn j=0 row is host-set to (-1)^m so stage E adds the Nyquist term for free.

gmf and the DFT matrices are tiny 3x3-derived constants computed host-side
(float64) and replicated to all cores; no cross-device communication.
All DRAM tensors are host-packed in the exact SBUF tile layout so every DMA is
128 large contiguous descriptors.
"""
import os

import ml_dtypes
import numpy as np

import concourse.bacc as bacc
import concourse.mybir as mybir
import concourse.tile as tile
from concourse.bass_utils import run_bass_kernel_spmd

F32 = mybir.dt.float32
F32R = mybir.dt.float32r
BF16 = mybir.dt.bfloat16

HP = 768          # padded grid
J = 385           # rfft half length (768//2+1)
JP = 386          # padded to even for fp32r free-dim constraint
NS = 8            # samples per core
NCORES = 8

LAST_EXEC_NS = None
LAST_RESULTS = None


def _build_constants(w):
    """Host-side constants (float64 -> float32), packed in SBUF tile layout."""
    w = np.asarray(w, np.float64)
    hm1 = np.zeros((HP, HP)); hm1[:3, :3] = w
    gm1f = 1.0 / np.fft.rfft2(hm1)
    gm2f = np.roll(gm1f[::-1, :], shift=1, axis=0)
    gm3f = np.roll(gm1f[:, ::-1], shift=1, axis=1)
    gm4f = np.roll(gm3f[::-1, :], shift=1, axis=0)
    gmf = (gm1f * gm2f) * (gm3f * gm4f)          # (768, 385) complex

    h = np.arange(512)
    k = np.arange(J)
    ph = np.exp(-2j * np.pi * (np.outer(h + 128, k) % HP) / HP)   # (512,385)
    c1 = np.zeros((4, 512, JP))
    c1[0, :, :J] = ph.real            # C1 == W2 (same 512x385 phase table)
    c1[1, :, :J] = ph.imag
    c1[2] = c1[0] + c1[1]             # Karatsuba sum table
    c1[3] = c1[0] - c1[1]             # Karatsuba dif table (mirror chunks)

    # pq with IDENTITY row map (layout row r holds true spectral row r; the
    # mirror chunks of stage B are emitted k'-reversed to make this so)
    r = np.arange(HP)
    pq = np.zeros((2, HP, JP))
    pq[0, :, :J] = gmf.real
    pq[1, :, :J] = gmf.imag
    # Nyquist pack: j=384 column becomes the r-symmetrized filter so that
    # T[:,384] is Hermitian in r and ifft of it is real (= U384re).
    gsym = (gmf[:, 384] + np.conj(gmf[(HP - r) % HP, 384])) / 2.0
    pq[0, :, 384] = gsym.real
    pq[1, :, 384] = gsym.imag

    # radix-2 DIF split tables for stage D: U[2n'] = sum_s (T[s]+T[s+384]) ae,
    # U[2n'+1] = sum_s (T[s]-T[s+384]) ao
    s = np.arange(384)
    npr = np.arange(384)
    ae = np.exp(2j * np.pi * np.outer(s, npr) / 384.0) / (HP * HP)
    ao = np.exp(2j * np.pi * (np.outer(s, 2 * npr + 1) % HP) / HP) / (HP * HP)
    a2 = np.zeros((3, 2, 384, 384))
    for se, tab in enumerate((ae, ao)):
        a2[0, se] = tab.real
        a2[1, se] = tab.imag
        a2[2, se] = tab.real + tab.imag

    m = np.arange(HP)
    j = np.arange(J)
    wj = np.where((j == 0) | (j == 384), 1.0, 2.0)
    ang = 2 * np.pi * (np.outer(j, m) % HP) / HP
    bre = wj[:, None] * np.cos(ang)              # (385, 768)
    bimn = -wj[:, None] * np.sin(ang)
    bmat = np.stack([bre[:384], bimn[:384]])     # (2, 384, 768)
    bmat[1, 0, :] = np.cos(np.pi * m)            # packed Nyquist row: (-1)^m

    f = np.float32
    bf = ml_dtypes.bfloat16
    return {
        # packed to SBUF layouts: leading dim = partition
        "c1": np.ascontiguousarray(c1.reshape(4, 4, 128, JP).transpose(2, 0, 1, 3)).astype(bf),
        "pq": np.ascontiguousarray(pq.reshape(2, 6, 128, JP).transpose(2, 0, 1, 3)).astype(bf),
        "a2": np.ascontiguousarray(a2.reshape(3, 2, 3, 128, 384).transpose(3, 0, 1, 2, 4)).astype(bf),
        "bmat": np.ascontiguousarray(bmat.reshape(2, 3, 128, HP).transpose(2, 0, 1, 3)).astype(bf),
    }


def _build_program(ns=NS):
    nc = bacc.Bacc("TRN2", target_bir_lowering=False, debug=False,
                   num_devices=NCORES)
    x_ext = nc.declare_dram_parameter("x", [ns, 128, 4, 512], BF16, isOutput=False)
    y_ext = nc.declare_dram_parameter("y", [ns, 128, 6, HP], F32, isOutput=True)
    c1_ext = nc.declare_dram_parameter("c1", [128, 4, 4, JP], BF16, isOutput=False)
    pq_ext = nc.declare_dram_parameter("pq", [128, 2, 6, JP], BF16, isOutput=False)
    a2_ext = nc.declare_dram_parameter("a2", [128, 3, 2, 3, 384], BF16, isOutput=False)
    bmat_ext = nc.declare_dram_parameter("bmat", [128, 2, 3, HP], BF16, isOutput=False)

    MUL = mybir.AluOpType.mult
    ADD = mybir.AluOpType.add
    SUB = mybir.AluOpType.subtract

    TRE, TIM, TSUM = 0, 1, 2

    with tile.TileContext(nc) as tc:
        with tc.tile_pool(name="const", bufs=1) as cpool, \
             tc.tile_pool(name="data", bufs=2) as dpool, \
             tc.tile_pool(name="xin", bufs=1) as xpool, \
             tc.tile_pool(name="yout", bufs=2) as ypool, \
             tc.tile_pool(name="scr", bufs=2) as spool, \
             tc.tile_pool(name="psA", bufs=2, space="PSUM") as ppa, \
             tc.tile_pool(name="psBD", bufs=4, space="PSUM") as ppb, \
             tc.tile_pool(name="psE", bufs=2, space="PSUM") as ppe:

            # sample-0 input first so stage A can start during const loads
            xts = []
            xt0 = xpool.tile([128, 4, 512], BF16, tag="x")
            nc.sync.dma_start(out=xt0[:], in_=x_ext[0])
            xts.append(xt0)

            c1_t = cpool.tile([128, 4, 4, JP], BF16, tag="c1")
            nc.sync.dma_start(out=c1_t[:, 0:1], in_=c1_ext[:, 0:1])
            nc.sync.dma_start(out=c1_t[:, 1:2], in_=c1_ext[:, 1:2])
            nc.sync.dma_start(out=c1_t[:, 2:4], in_=c1_ext[:, 2:4])
            pq_t = cpool.tile([128, 2, 6, JP], BF16, tag="pq")
            nc.sync.dma_start(out=pq_t[:], in_=pq_ext[:])
            a2_t = cpool.tile([128, 3, 2, 3, 384], BF16, tag="a2")
            nc.sync.dma_start(out=a2_t[:], in_=a2_ext[:])
            b_t = cpool.tile([128, 2, 3, HP], BF16, tag="bmat")
            nc.sync.dma_start(out=b_t[:], in_=bmat_ext[:])

            def mm(ps, lhsT, rhs, start, stop):
                nc.tensor.matmul(ps, lhsT=lhsT, rhs=rhs, start=start, stop=stop)

            def stage_e(eb, eut):
                for nch in range(6):
                    nsl = slice(nch * 128, nch * 128 + 128)
                    ytc = ypool.tile([128, HP], F32, tag="y")
                    for mh in range(2):
                        msl = slice(mh * 384, mh * 384 + 384)
                        ps_y = ppe.tile([128, 384], F32, tag="psE")
                        for jc in range(3):
                            mm(ps_y[:], eut[:, 0, jc, nsl], b_t[:, 0, jc, msl], jc == 0, False)
                        for jc in range(3):
                            mm(ps_y[:], eut[:, 1, jc, nsl], b_t[:, 1, jc, msl], False, jc == 2)
                        nc.scalar.copy(out=ytc[:, msl], in_=ps_y[:])
                        nc.sync.dma_start(out=y_ext[eb, :, nch, msl], in_=ytc[:, msl])

            prev_e = None
            for b in range(ns):
                xt = xts[b]
                if b + 1 < ns:   # prefetch next sample
                    nxt = xpool.tile([128, 4, 512], BF16, tag="x")
                    nc.sync.dma_start(out=nxt[:], in_=x_ext[b + 1])
                    xts.append(nxt)

                s1 = dpool.tile([128, 3, 4, JP], BF16, tag="s1")
                s1r = dpool.tile([128, 3, 4, 384], BF16, tag="s1r")
                tmat = dpool.tile([128, 2, 6, JP], BF16, tag="tmat")
                vmat = dpool.tile([128, 3, 2, 3, JP], BF16, tag="vmat")
                ut = dpool.tile([128, 2, 3, HP], BF16, tag="ut")

                # ---- stage A ----
                for comp in range(2):
                    for wc in range(4):
                        ps = ppa.tile([128, JP], F32, tag="psA")
                        for hc in range(4):
                            mm(ps[:], xt[:, hc, wc * 128:(wc + 1) * 128],
                               c1_t[:, comp, hc, :], hc == 0, hc == 3)
                        nc.scalar.copy(out=s1[:, comp, wc, :], in_=ps[:])
                        # reversed copy for the R-ascending mirror chunks:
                        # s1r[., i] = S1[., 384-i]
                        nc.scalar.copy(out=s1r[:, comp, wc, :], in_=ps[:, 384:0:-1])
                # Karatsuba lhsT sums (per w-chunk so B's M3 can start early)
                for wc in range(4):
                    nc.vector.tensor_tensor(out=s1[:, 2, wc, :], in0=s1[:, 0, wc, :],
                                            in1=s1[:, 1, wc, :], op=ADD)
                    nc.vector.tensor_tensor(out=s1r[:, 2, wc, :], in0=s1r[:, 0, wc, :],
                                            in1=s1r[:, 1, wc, :], op=ADD)

                # ---- stages B + C, 6 chunks of T ----
                for c in range(6):
                    mirror = c >= 3
                    sm = s1r if mirror else s1
                    ksl = slice((c - 3) * 128, (c - 3) * 128 + 128) if mirror \
                        else slice(c * 128, c * 128 + 128)
                    ps1 = ppb.tile([128, JP], F32, tag="psBD")
                    ps2 = ppb.tile([128, JP], F32, tag="psBD")
                    ps3 = ppb.tile([128, JP], F32, tag="psBD")
                    for wc in range(4):
                        mm(ps1[:], sm[:, 0, wc, ksl], c1_t[:, 0, wc, :], wc == 0, wc == 3)
                    for wc in range(4):
                        mm(ps2[:], sm[:, 1, wc, ksl], c1_t[:, 1, wc, :], wc == 0, wc == 3)
                    for wc in range(4):
                        mm(ps3[:], sm[:, 2, wc, ksl],
                           c1_t[:, 3 if mirror else 2, wc, :], wc == 0, wc == 3)
                    # Karatsuba recombine: xre = M1 -+ M2 ; xim = M3 - M1 -+ M2
                    # (M2 staged through ACT since DVE reads only one PSUM input)
                    xre = spool.tile([128, JP], BF16, tag="xre")
                    xim = spool.tile([128, JP], BF16, tag="xim")
                    nc.scalar.copy(out=xim[:], in_=ps2[:])
                    nc.vector.tensor_tensor(out=xre[:], in0=ps1[:], in1=xim[:],
                                            op=ADD if mirror else SUB)
                    nc.vector.scalar_tensor_tensor(
                        out=xim[:], in0=xim[:], scalar=-2.0 if mirror else 2.0,
                        in1=xre[:], op0=MUL, op1=ADD)
                    nc.vector.tensor_tensor(out=xim[:], in0=ps3[:], in1=xim[:],
                                            op=SUB)

                    # ---- stage C on this chunk ----
                    t_re = tmat[:, TRE, c, :]
                    t_im = tmat[:, TIM, c, :]
                    pA = spool.tile([128, JP], BF16, tag="pA")
                    pB = spool.tile([128, JP], BF16, tag="pB")
                    nc.vector.tensor_tensor(out=t_re, in0=pq_t[:, 0, c, :], in1=xre[:], op=MUL)
                    nc.vector.tensor_tensor(out=pA[:], in0=pq_t[:, 1, c, :], in1=xim[:], op=MUL)
                    nc.vector.tensor_tensor(out=t_re, in0=t_re, in1=pA[:],
                                            op=ADD if mirror else SUB)
                    nc.vector.tensor_tensor(out=t_im, in0=pq_t[:, 1, c, :], in1=xre[:], op=MUL)
                    nc.vector.tensor_tensor(out=pB[:], in0=pq_t[:, 0, c, :], in1=xim[:], op=MUL)
                    nc.vector.tensor_tensor(out=t_im, in0=t_im, in1=pB[:],
                                            op=SUB if mirror else ADD)
                    # Tmix: pack Nyquist column into j=0 (per-chunk so stage D
                    # pipelines per tmat chunk instead of waiting for all of C)
                    nc.vector.tensor_tensor(out=t_re[:, 0:1], in0=t_re[:, 0:1],
                                            in1=t_im[:, 384:385], op=SUB)
                    nc.vector.tensor_tensor(out=t_im[:, 0:1], in0=t_im[:, 0:1],
                                            in1=t_re[:, 384:385], op=ADD)
                    # radix-2 DIF fold once the (sc, sc+3) chunk pair is done:
                    # V+[s] = T[s] + T[s+384], V-[s] = T[s] - T[s+384]
                    if mirror:
                        sc = c - 3
                        for cmp in range(2):
                            nc.vector.tensor_tensor(
                                out=vmat[:, cmp, 0, sc, :], in0=tmat[:, cmp, sc, :],
                                in1=tmat[:, cmp, c, :], op=ADD)
                            nc.vector.tensor_tensor(
                                out=vmat[:, cmp, 1, sc, :], in0=tmat[:, cmp, sc, :],
                                in1=tmat[:, cmp, c, :], op=SUB)
                        for se in range(2):
                            nc.vector.tensor_tensor(
                                out=vmat[:, 2, se, sc, :], in0=vmat[:, 0, se, sc, :],
                                in1=vmat[:, 1, se, sc, :], op=ADD)

                # stage E of the previous sample goes here in program order so
                # the PE has ready matmuls while this sample's C chains drain
                if prev_e is not None:
                    stage_e(*prev_e)

                # ---- stage D: radix-2 split, 9 matmuls per (jc, parity) ----
                for jc in range(3):
                    jsl = slice(jc * 128, jc * 128 + 128)
                    for se in range(2):
                        nsl = slice(se * 384, se * 384 + 384)
                        pm1 = ppb.tile([128, 384], F32, tag="psBD")
                        pm2 = ppb.tile([128, 384], F32, tag="psBD")
                        pm3 = ppb.tile([128, 384], F32, tag="psBD")
                        for sc in range(3):
                            mm(pm1[:], vmat[:, 0, se, sc, jsl], a2_t[:, 0, se, sc, :], sc == 0, sc == 2)
                        for sc in range(3):
                            mm(pm2[:], vmat[:, 1, se, sc, jsl], a2_t[:, 1, se, sc, :], sc == 0, sc == 2)
                        for sc in range(3):
                            mm(pm3[:], vmat[:, 2, se, sc, jsl], a2_t[:, 2, se, sc, :], sc == 0, sc == 2)
                        # Ure = M1 - M2 ; Uim = M3 - M1 - M2
                        # (M2 staged into the Uim slot via ACT; one-PSUM rule)
                        u0 = ut[:, 0, jc, nsl]
                        u1 = ut[:, 1, jc, nsl]
                        nc.scalar.copy(out=u1, in_=pm2[:])
                        nc.vector.tensor_tensor(out=u0, in0=pm1[:], in1=u1, op=SUB)
                        nc.vector.scalar_tensor_tensor(
                            out=u1, in0=u1, scalar=2.0, in1=u0, op0=MUL, op1=ADD)
                        nc.vector.tensor_tensor(out=u1, in0=pm3[:], in1=u1, op=SUB)

                prev_e = (b, ut)

            stage_e(*prev_e)

    nc.compile()
    return nc


_PROGRAM_CACHE = {}


def kernel(x, w, trace=False):
    global LAST_EXEC_NS, LAST_RESULTS
    x = np.asarray(x, np.float32)
    B = x.shape[0]
    # pack to the SBUF tile layout: x_dev[b, p, c, w] = x[b, c*128+p, w]
    x_dev = np.ascontiguousarray(
        x.reshape(B, 4, 128, 512).transpose(0, 2, 1, 3)).astype(ml_dtypes.bfloat16)
    consts = _build_constants(w)
    if NS not in _PROGRAM_CACHE:
        _PROGRAM_CACHE[NS] = _build_program(NS)
    nc = _PROGRAM_CACHE[NS]
    in_maps = []
    for core in range(NCORES):
        m = {"x": x_dev[core * NS:(core + 1) * NS]}
        m.update(consts)
        in_maps.append(m)
    if trace:
        os.environ.pop("BASS_NEVER_TRACE", None)
        res = run_bass_kernel_spmd(nc, in_maps, list(range(NCORES)), trace=True)
    else:
        # profiling needs the antenv NTFF shim; never let a stray BASS_TRACE
        # env var route us down that path during plain runs
        os.environ["BASS_NEVER_TRACE"] = "1"
        try:
            res = run_bass_kernel_spmd(nc, in_maps, list(range(NCORES)), trace=False)
        finally:
            os.environ.pop("BASS_NEVER_TRACE", None)
    LAST_EXEC_NS = res.exec_time_ns
    LAST_RESULTS = res
    # unshard: y_dev[b, p, c, m] -> row q = c*128+p; stage D's radix-2 DIF
    # interleave means q < 384 holds even output rows, q >= 384 odd rows
    y_dev = np.concatenate([res.results[i]["y"] for i in range(NCORES)], axis=0)
    yq = y_dev.transpose(0, 2, 1, 3).reshape(B, HP, HP)
    q = np.arange(HP)
    perm = np.where(q < 384, 2 * q, 2 * (q - 384) + 1)
    y = np.empty_like(yq)
    y[:, perm, :] = yq
    return np.ascontiguousarray(y, np.float32)


# revision 21
# speedup vs baseline: 1.1128x; 1.0143x over previous
"""Trainium2 Bass kernel for nn_DeconvDft2dLayer.

y = irfft2(gmf * rfft2(pad(x)))  with x (64,512,512), w (3,3), y (64,768,768).

Strategy: data-parallel over batch (8 samples per NeuronCore). Per sample the
FFTs are evaluated as DFT matmuls on the tensor engine (fp32r, full rate):

  A : S1^T[w,k] = sum_h x[h,w] W2[h,k]            k in [0,385)   (fft-H, halved
      via Hermitian symmetry of the real input)
  B : X[k,j]    = sum_w S1[k,w] C1[w,j]  (Karatsuba: M1=S1re@C1re, M2=S1im@C1im,
      M3=(S1re+S1im)@(C1re+-C1im); re=M1-+M2, im=M3-re-+2*M2; mirror chunks use
      conj(C1) which flips the +- signs)
  C : T[r,j] = gmf[rho(r)] * X  -- elementwise on Vector+GpSimd
      (rho(r) = r for r<384, 1151-r otherwise)
  D : U^T[j,n]  = sum_r T[r,j] Atil[r,n]  (Karatsuba again)
  E : y[n,m]    = sum_j Ure[j,n] Bre[j,m] + Uim[j,n] Bimn[j,m]
      Bre = w_j cos(2 pi j m/768), Bimn = -w_j sin(2 pi j m/768)

Nyquist handling (no dedicated matmuls): column j=384 of the pq table holds the
r-Hermitian-symmetrized filter, so T[:,384] is Hermitian and ifft(T[:,384]) =
Re(U[:,384]).  After stage C the j=0 column (whose U is real since the j=0
filter column is Hermitian) is mixed as Tmix = T[:,0] + i*T[:,384]; stage D then
lands U[:,0] in Ure row 0 and U384re in the (otherwise zero) Uim row 0, and the
Bimn j=0 row is host-set to (-1)^m so stage E adds the Nyquist term for free.

gmf and the DFT matrices are tiny 3x3-derived constants computed host-side
(float64) and replicated to all cores; no cross-device communication.
All DRAM tensors are host-packed in the exact SBUF tile layout so every DMA is
128 large contiguous descriptors.
"""
import os

import ml_dtypes
import numpy as np

import concourse.bacc as bacc
import concourse.mybir as mybir
import concourse.tile as tile
from concourse.bass_utils import run_bass_kernel_spmd

F32 = mybir.dt.float32
F32R = mybir.dt.float32r
BF16 = mybir.dt.bfloat16

HP = 768          # padded grid
J = 385           # rfft half length (768//2+1)
JP = 386          # padded to even for fp32r free-dim constraint
NS = 8            # samples per core
NCORES = 8

LAST_EXEC_NS = None
LAST_RESULTS = None


def _build_constants(w):
    """Host-side constants (float64 -> float32), packed in SBUF tile layout."""
    w = np.asarray(w, np.float64)
    hm1 = np.zeros((HP, HP)); hm1[:3, :3] = w
    gm1f = 1.0 / np.fft.rfft2(hm1)
    gm2f = np.roll(gm1f[::-1, :], shift=1, axis=0)
    gm3f = np.roll(gm1f[:, ::-1], shift=1, axis=1)
    gm4f = np.roll(gm3f[::-1, :], shift=1, axis=0)
    gmf = (gm1f * gm2f) * (gm3f * gm4f)          # (768, 385) complex

    h = np.arange(512)
    k = np.arange(J)
    ph = np.exp(-2j * np.pi * (np.outer(h + 128, k) % HP) / HP)   # (512,385)
    c1 = np.zeros((4, 512, JP))
    c1[0, :, :J] = ph.real            # C1 == W2 (same 512x385 phase table)
    c1[1, :, :J] = ph.imag
    c1[2] = c1[0] + c1[1]             # Karatsuba sum table
    c1[3] = c1[0] - c1[1]             # Karatsuba dif table (mirror chunks)

    # pq with IDENTITY row map (layout row r holds true spectral row r; the
    # mirror chunks of stage B are emitted k'-reversed to make this so)
    r = np.arange(HP)
    pq = np.zeros((2, HP, JP))
    pq[0, :, :J] = gmf.real
    pq[1, :, :J] = gmf.imag
    # Nyquist pack: j=384 column becomes the r-symmetrized filter so that
    # T[:,384] is Hermitian in r and ifft of it is real (= U384re).
    gsym = (gmf[:, 384] + np.conj(gmf[(HP - r) % HP, 384])) / 2.0
    pq[0, :, 384] = gsym.real
    pq[1, :, 384] = gsym.imag

    # radix-2 DIF split tables for stage D: U[2n'] = sum_s (T[s]+T[s+384]) ae,
    # U[2n'+1] = sum_s (T[s]-T[s+384]) ao
    s = np.arange(384)
    npr = np.arange(384)
    ae = np.exp(2j * np.pi * np.outer(s, npr) / 384.0) / (HP * HP)
    ao = np.exp(2j * np.pi * (np.outer(s, 2 * npr + 1) % HP) / HP) / (HP * HP)
    a2 = np.zeros((3, 2, 384, 384))
    for se, tab in enumerate((ae, ao)):
        a2[0, se] = tab.real
        a2[1, se] = tab.imag
        a2[2, se] = tab.real + tab.imag

    m = np.arange(HP)
    j = np.arange(J)
    wj = np.where((j == 0) | (j == 384), 1.0, 2.0)
    ang = 2 * np.pi * (np.outer(j, m) % HP) / HP
    bre = wj[:, None] * np.cos(ang)              # (385, 768)
    bimn = -wj[:, None] * np.sin(ang)
    bmat = np.stack([bre[:384], bimn[:384]])     # (2, 384, 768)
    bmat[1, 0, :] = np.cos(np.pi * m)            # packed Nyquist row: (-1)^m

    f = np.float32
    bf = ml_dtypes.bfloat16
    return {
        # packed to SBUF layouts: leading dim = partition
        "c1": np.ascontiguousarray(c1.reshape(4, 4, 128, JP).transpose(2, 0, 1, 3)).astype(bf),
        "pq": np.ascontiguousarray(pq.reshape(2, 6, 128, JP).transpose(2, 0, 1, 3)).astype(bf),
        "a2": np.ascontiguousarray(a2.reshape(3, 2, 3, 128, 384).transpose(3, 0, 1, 2, 4)).astype(bf),
        "bmat": np.ascontiguousarray(bmat.reshape(2, 3, 128, HP).transpose(2, 0, 1, 3)).astype(bf),
    }


def _build_program(ns=NS):
    nc = bacc.Bacc("TRN2", target_bir_lowering=False, debug=False,
                   num_devices=NCORES)
    x_ext = nc.declare_dram_parameter("x", [ns, 128, 4, 512], BF16, isOutput=False)
    y_ext = nc.declare_dram_parameter("y", [ns, 128, 6, HP], F32, isOutput=True)
    c1_ext = nc.declare_dram_parameter("c1", [128, 4, 4, JP], BF16, isOutput=False)
    pq_ext = nc.declare_dram_parameter("pq", [128, 2, 6, JP], BF16, isOutput=False)
    a2_ext = nc.declare_dram_parameter("a2", [128, 3, 2, 3, 384], BF16, isOutput=False)
    bmat_ext = nc.declare_dram_parameter("bmat", [128, 2, 3, HP], BF16, isOutput=False)

    MUL = mybir.AluOpType.mult
    ADD = mybir.AluOpType.add
    SUB = mybir.AluOpType.subtract

    TRE, TIM, TSUM = 0, 1, 2

    with tile.TileContext(nc) as tc:
        with tc.tile_pool(name="const", bufs=1) as cpool, \
             tc.tile_pool(name="data", bufs=2) as dpool, \
             tc.tile_pool(name="xin", bufs=1) as xpool, \
             tc.tile_pool(name="yout", bufs=3) as ypool, \
             tc.tile_pool(name="scr", bufs=4) as spool, \
             tc.tile_pool(name="psA", bufs=2, space="PSUM") as ppa, \
             tc.tile_pool(name="psBD", bufs=4, space="PSUM") as ppb, \
             tc.tile_pool(name="psE", bufs=2, space="PSUM") as ppe:

            # sample-0 input first so stage A can start during const loads
            xts = []
            xt0 = xpool.tile([128, 4, 512], BF16, tag="x")
            nc.sync.dma_start(out=xt0[:], in_=x_ext[0])
            xts.append(xt0)

            c1_t = cpool.tile([128, 4, 4, JP], BF16, tag="c1")
            nc.sync.dma_start(out=c1_t[:, 0:1], in_=c1_ext[:, 0:1])
            nc.sync.dma_start(out=c1_t[:, 1:2], in_=c1_ext[:, 1:2])
            nc.sync.dma_start(out=c1_t[:, 2:4], in_=c1_ext[:, 2:4])
            pq_t = cpool.tile([128, 2, 6, JP], BF16, tag="pq")
            nc.sync.dma_start(out=pq_t[:], in_=pq_ext[:])
            a2_t = cpool.tile([128, 3, 2, 3, 384], BF16, tag="a2")
            nc.sync.dma_start(out=a2_t[:], in_=a2_ext[:])
            b_t = cpool.tile([128, 2, 3, HP], BF16, tag="bmat")
            nc.sync.dma_start(out=b_t[:], in_=bmat_ext[:])

            def mm(ps, lhsT, rhs, start, stop):
                nc.tensor.matmul(ps, lhsT=lhsT, rhs=rhs, start=start, stop=stop)

            def stage_e(eb, eut):
                for nch in range(6):
                    nsl = slice(nch * 128, nch * 128 + 128)
                    ytc = ypool.tile([128, HP], F32, tag="y")
                    for mh in range(2):
                        msl = slice(mh * 384, mh * 384 + 384)
                        ps_y = ppe.tile([128, 384], F32, tag="psE")
                        for jc in range(3):
                            mm(ps_y[:], eut[:, 0, jc, nsl], b_t[:, 0, jc, msl], jc == 0, False)
                        for jc in range(3):
                            mm(ps_y[:], eut[:, 1, jc, nsl], b_t[:, 1, jc, msl], False, jc == 2)
                        nc.scalar.copy(out=ytc[:, msl], in_=ps_y[:])
                        nc.sync.dma_start(out=y_ext[eb, :, nch, msl], in_=ytc[:, msl])

            prev_e = None
            for b in range(ns):
                xt = xts[b]
                if b + 1 < ns:   # prefetch next sample
                    nxt = xpool.tile([128, 4, 512], BF16, tag="x")
                    nc.sync.dma_start(out=nxt[:], in_=x_ext[b + 1])
                    xts.append(nxt)

                s1 = dpool.tile([128, 3, 4, JP], BF16, tag="s1")
                s1r = dpool.tile([128, 3, 4, 384], BF16, tag="s1r")
                tmat = dpool.tile([128, 2, 6, JP], BF16, tag="tmat")
                vmat = dpool.tile([128, 3, 2, 3, JP], BF16, tag="vmat")
                ut = dpool.tile([128, 2, 3, HP], BF16, tag="ut")

                # ---- stage A ----
                for comp in range(2):
                    for wc in range(4):
                        ps = ppa.tile([128, JP], F32, tag="psA")
                        for hc in range(4):
                            mm(ps[:], xt[:, hc, wc * 128:(wc + 1) * 128],
                               c1_t[:, comp, hc, :], hc == 0, hc == 3)
                        nc.scalar.copy(out=s1[:, comp, wc, :], in_=ps[:])
                        # reversed copy for the R-ascending mirror chunks:
                        # s1r[., i] = S1[., 384-i]
                        nc.scalar.copy(out=s1r[:, comp, wc, :], in_=ps[:, 384:0:-1])
                # Karatsuba lhsT sums (per w-chunk so B's M3 can start early)
                for wc in range(4):
                    nc.vector.tensor_tensor(out=s1[:, 2, wc, :], in0=s1[:, 0, wc, :],
                                            in1=s1[:, 1, wc, :], op=ADD)
                    nc.vector.tensor_tensor(out=s1r[:, 2, wc, :], in0=s1r[:, 0, wc, :],
                                            in1=s1r[:, 1, wc, :], op=ADD)

                # ---- stages B + C, 6 chunks of T ----
                for c in range(6):
                    mirror = c >= 3
                    sm = s1r if mirror else s1
                    ksl = slice((c - 3) * 128, (c - 3) * 128 + 128) if mirror \
                        else slice(c * 128, c * 128 + 128)
                    ps1 = ppb.tile([128, JP], F32, tag="psBD")
                    ps2 = ppb.tile([128, JP], F32, tag="psBD")
                    ps3 = ppb.tile([128, JP], F32, tag="psBD")
                    for wc in range(4):
                        mm(ps1[:], sm[:, 0, wc, ksl], c1_t[:, 0, wc, :], wc == 0, wc == 3)
                    for wc in range(4):
                        mm(ps2[:], sm[:, 1, wc, ksl], c1_t[:, 1, wc, :], wc == 0, wc == 3)
                    for wc in range(4):
                        mm(ps3[:], sm[:, 2, wc, ksl],
                           c1_t[:, 3 if mirror else 2, wc, :], wc == 0, wc == 3)
                    # Karatsuba recombine: xre = M1 -+ M2 ; xim = M3 - M1 -+ M2
                    # (M2 staged through ACT since DVE reads only one PSUM input)
                    xre = spool.tile([128, JP], BF16, tag="xre")
                    xim = spool.tile([128, JP], BF16, tag="xim")
                    nc.scalar.copy(out=xim[:], in_=ps2[:])
                    nc.vector.tensor_tensor(out=xre[:], in0=ps1[:], in1=xim[:],
                                            op=ADD if mirror else SUB)
                    nc.vector.scalar_tensor_tensor(
                        out=xim[:], in0=xim[:], scalar=-2.0 if mirror else 2.0,
                        in1=xre[:], op0=MUL, op1=ADD)
                    nc.vector.tensor_tensor(out=xim[:], in0=ps3[:], in1=xim[:],
                                            op=SUB)

                    # ---- stage C on this chunk ----
                    t_re = tmat[:, TRE, c, :]
                    t_im = tmat[:, TIM, c, :]
                    pA = spool.tile([128, JP], BF16, tag="pA")
                    pB = spool.tile([128, JP], BF16, tag="pB")
                    nc.vector.tensor_tensor(out=t_re, in0=pq_t[:, 0, c, :], in1=xre[:], op=MUL)
                    nc.vector.tensor_tensor(out=pA[:], in0=pq_t[:, 1, c, :], in1=xim[:], op=MUL)
                    nc.vector.tensor_tensor(out=t_re, in0=t_re, in1=pA[:],
                                            op=ADD if mirror else SUB)
                    nc.vector.tensor_tensor(out=t_im, in0=pq_t[:, 1, c, :], in1=xre[:], op=MUL)
                    nc.vector.tensor_tensor(out=pB[:], in0=pq_t[:, 0, c, :], in1=xim[:], op=MUL)
                    nc.vector.tensor_tensor(out=t_im, in0=t_im, in1=pB[:],
                                            op=SUB if mirror else ADD)
                    # Tmix: pack Nyquist column into j=0 (per-chunk so stage D
                    # pipelines per tmat chunk instead of waiting for all of C)
                    nc.vector.tensor_tensor(out=t_re[:, 0:1], in0=t_re[:, 0:1],
                                            in1=t_im[:, 384:385], op=SUB)
                    nc.vector.tensor_tensor(out=t_im[:, 0:1], in0=t_im[:, 0:1],
                                            in1=t_re[:, 384:385], op=ADD)
                    # radix-2 DIF fold once the (sc, sc+3) chunk pair is done:
                    # V+[s] = T[s] + T[s+384], V-[s] = T[s] - T[s+384]
                    if mirror:
                        sc = c - 3
                        for cmp in range(2):
                            nc.vector.tensor_tensor(
                                out=vmat[:, cmp, 0, sc, :], in0=tmat[:, cmp, sc, :],
                                in1=tmat[:, cmp, c, :], op=ADD)
                            nc.vector.tensor_tensor(
                                out=vmat[:, cmp, 1, sc, :], in0=tmat[:, cmp, sc, :],
                                in1=tmat[:, cmp, c, :], op=SUB)
                        for se in range(2):
                            nc.vector.tensor_tensor(
                                out=vmat[:, 2, se, sc, :], in0=vmat[:, 0, se, sc, :],
                                in1=vmat[:, 1, se, sc, :], op=ADD)

                # stage E of the previous sample goes here in program order so
                # the PE has ready matmuls while this sample's C chains drain
                if prev_e is not None:
                    stage_e(*prev_e)

                # ---- stage D: radix-2 split, 9 matmuls per (jc, parity) ----
                for jc in range(3):
                    jsl = slice(jc * 128, jc * 128 + 128)
                    for se in range(2):
                        nsl = slice(se * 384, se * 384 + 384)
                        pm1 = ppb.tile([128, 384], F32, tag="psBD")
                        pm2 = ppb.tile([128, 384], F32, tag="psBD")
                        pm3 = ppb.tile([128, 384], F32, tag="psBD")
                        for sc in range(3):
                            mm(pm1[:], vmat[:, 0, se, sc, jsl], a2_t[:, 0, se, sc, :], sc == 0, sc == 2)
                        for sc in range(3):
                            mm(pm2[:], vmat[:, 1, se, sc, jsl], a2_t[:, 1, se, sc, :], sc == 0, sc == 2)
                        for sc in range(3):
                            mm(pm3[:], vmat[:, 2, se, sc, jsl], a2_t[:, 2, se, sc, :], sc == 0, sc == 2)
                        # Ure = M1 - M2 ; Uim = M3 - M1 - M2
                        # (M2 staged into the Uim slot via ACT; one-PSUM rule)
                        u0 = ut[:, 0, jc, nsl]
                        u1 = ut[:, 1, jc, nsl]
                        nc.scalar.copy(out=u1, in_=pm2[:])
                        nc.vector.tensor_tensor(out=u0, in0=pm1[:], in1=u1, op=SUB)
                        nc.vector.scalar_tensor_tensor(
                            out=u1, in0=u1, scalar=2.0, in1=u0, op0=MUL, op1=ADD)
                        nc.vector.tensor_tensor(out=u1, in0=pm3[:], in1=u1, op=SUB)

                prev_e = (b, ut)

            stage_e(*prev_e)

    nc.compile()
    return nc


_PROGRAM_CACHE = {}


def kernel(x, w, trace=False):
    global LAST_EXEC_NS, LAST_RESULTS
    x = np.asarray(x, np.float32)
    B = x.shape[0]
    # pack to the SBUF tile layout: x_dev[b, p, c, w] = x[b, c*128+p, w]
    x_dev = np.ascontiguousarray(
        x.reshape(B, 4, 128, 512).transpose(0, 2, 1, 3)).astype(ml_dtypes.bfloat16)
    consts = _build_constants(w)
    if NS not in _PROGRAM_CACHE:
        _PROGRAM_CACHE[NS] = _build_program(NS)
    nc = _PROGRAM_CACHE[NS]
    in_maps = []
    for core in range(NCORES):
        m = {"x": x_dev[core * NS:(core + 1) * NS]}
        m.update(consts)
        in_maps.append(m)
    if trace:
        os.environ.pop("BASS_NEVER_TRACE", None)
        res = run_bass_kernel_spmd(nc, in_maps, list(range(NCORES)), trace=True)
    else:
        # profiling needs the antenv NTFF shim; never let a stray BASS_TRACE
        # env var route us down that path during plain runs
        os.environ["BASS_NEVER_TRACE"] = "1"
        try:
            res = run_bass_kernel_spmd(nc, in_maps, list(range(NCORES)), trace=False)
        finally:
            os.environ.pop("BASS_NEVER_TRACE", None)
    LAST_EXEC_NS = res.exec_time_ns
    LAST_RESULTS = res
    # unshard: y_dev[b, p, c, m] -> row q = c*128+p; stage D's radix-2 DIF
    # interleave means q < 384 holds even output rows, q >= 384 odd rows
    y_dev = np.concatenate([res.results[i]["y"] for i in range(NCORES)], axis=0)
    yq = y_dev.transpose(0, 2, 1, 3).reshape(B, HP, HP)
    q = np.arange(HP)
    perm = np.where(q < 384, 2 * q, 2 * (q - 384) + 1)
    y = np.empty_like(yq)
    y[:, perm, :] = yq
    return np.ascontiguousarray(y, np.float32)
